# revision 12
# baseline (speedup 1.0000x reference)
"""CenterLoss (segment_reduce) Trainium2 kernel — TensorE segment-sum version.

Math (faithful to the reference):
  preds = argmax_c logits[n, c, h, w]          (softmax is monotone -> skip it)
  per (n, cls): cnt = #pixels with preds==cls,
                S1 = sum over those pixels of sum_c x,
                S2 = sum over those pixels of sum_c x^2
  K = max(cnt,1)*C; sq_dev = max(S2 - S1^2/K, 0)
  loss = sum_cls mean_n( cnt>0 ? sqrt(sq_dev) : 0 )

Device strategy (8 cores, data-parallel over 16 units = (n, H-slab of 128)):
  Host pre-casts logits to bf16 (halves HBM traffic; bf16-argmax ties touch
  ~0.7% of pixels and shift the loss by ~4e-3 rel — within the 2e-2 gate).
  Each core takes 2 units as (128h partitions, [xb(19) | x^2(19) | 1] x 512w)
  "xs" tiles:
    ScalarE:  Square(xb) -> xs[:, 19:38, :]
    GpSimd:   ones row memset
    DVE:      m = max over c (pairwise bf16 tree, 2x mode)
              E = (xb == m) one-hot, ONE broadcast tensor_tensor (2x mode)
    TensorE:  per 4 w-columns, matmul(psum += E_grpT(128x76) @ xs_grp(128x156))
              PSUM-accumulated over the whole unit.  Off-diagonal (w,w')
              blocks are junk and ignored; diagonal blocks hold, per class,
              the per-channel sums of xb and x^2 plus the pixel count.
  The per-unit (76,156) PSUM is copied to SBUF (ScalarE) and DMA'd out;
  host extracts diagonal blocks and applies the final sqrt/mean formula.

  vs. the previous all-DVE version (57 scalar_tensor_tensor passes/chunk,
  ~250us): DVE now runs ~11us/chunk and the segment reduction rides the
  otherwise-idle TensorE.
"""

import numpy as np
import ml_dtypes


def _ensure_ntff_hook():
    """bass_utils' trace path imports antenv.axon_hooks, which this image
    lacks.  Install a shim backed by trn_agent_boot's ctypes hook so a
    BASS_TRACE=1 environment doesn't crash the run (and tracing works)."""
    import sys
    import types

    try:
        import antenv.axon_hooks  # noqa: F401
        return
    except ImportError:
        pass
    try:
        from trn_agent_boot.trn_boot import _ntff_profile_via_ctypes

        hook = _ntff_profile_via_ctypes("/opt/axon/libaxon_pjrt.so")
    except Exception:
        hook = None
    mod = types.ModuleType("antenv.axon_hooks")
    mod.get_axon_ntff_profile_hook = lambda: hook
    mod.set_axon_ntff_profile_hook = lambda h: None
    sys.modules["antenv.axon_hooks"] = mod

N, C, H, W = 4, 19, 512, 1024
NCORES = 8
SLABS = 4                 # H split into 4 slabs of 128 partitions
P = H // SLABS            # 128
UNITS = [(n, s) for n in range(N) for s in range(SLABS)]   # 16 units
UPC = len(UNITS) // NCORES                                  # 2 units per core
WCHUNK = 512
NCHUNKS = W // WCHUNK
WG = 4                    # w-columns fused per matmul (grouped-E stationary)
NQ = 2 * C + 1            # 39 xs columns: [xb-chans | sq-chans | count]
MROWS = C * WG            # 76 psum partitions, m = c*WG + jm
NCOLS = NQ * WG           # 156 psum columns, n = jn*NQ + q; keep jm == jn

_CACHE = {}


def _build_nc():
    from contextlib import ExitStack

    import concourse.tile as tile
    from concourse import bacc, mybir

    f32 = mybir.dt.float32
    bf16 = mybir.dt.bfloat16
    Alu = mybir.AluOpType
    Act = mybir.ActivationFunctionType

    nc = bacc.Bacc("TRN2", target_bir_lowering=False, debug=False)
    # Host pre-arranges each core's shard as (unit, wchunk, h, c, w) bf16 so
    # one chunk load is 128 fully contiguous 19.5 KB runs.
    x_d = nc.dram_tensor(
        "x", [UPC, NCHUNKS, P, C, WCHUNK], bf16, kind="ExternalInput"
    ).ap()
    out_d = nc.dram_tensor(
        "stats", [UPC, MROWS, NCOLS], f32, kind="ExternalOutput"
    ).ap()

    with tile.TileContext(nc) as tc, ExitStack() as ctx:
        xpool = ctx.enter_context(tc.tile_pool(name="xs", bufs=2))
        epool = ctx.enter_context(tc.tile_pool(name="eq", bufs=2))
        tpool = ctx.enter_context(tc.tile_pool(name="tree", bufs=2))
        ppool = ctx.enter_context(tc.tile_pool(name="psum", bufs=2, space="PSUM"))
        spool = ctx.enter_context(tc.tile_pool(name="sb_out", bufs=2))

        def tree(src, tag):
            """Pairwise max-reduce the C=19 rows of 3-dim AP `src`
            (P, 19, WCHUNK) along the row dim via contiguous bf16
            tensor_tensor ops (2x DVE mode); leftovers (src row 18,
            level-1 row 8) fold in at the end.  Returns a (P, WCHUNK) AP."""
            assert C == 19
            t = tpool.tile([P, 10, WCHUNK], bf16, tag="tree", name=f"tree_{tag}")
            tt = nc.vector.tensor_tensor
            op = Alu.max
            tt(out=t[:, 0:9, :], in0=src[:, 0:9, :], in1=src[:, 9:18, :], op=op)
            tt(out=t[:, 0:4, :], in0=t[:, 0:4, :], in1=t[:, 4:8, :], op=op)
            tt(out=t[:, 0:2, :], in0=t[:, 0:2, :], in1=t[:, 2:4, :], op=op)
            tt(out=t[:, 0, :], in0=t[:, 0, :], in1=t[:, 1, :], op=op)
            tt(out=t[:, 0, :], in0=t[:, 0, :], in1=t[:, 8, :], op=op)
            tt(out=t[:, 9, :], in0=t[:, 0, :], in1=src[:, 18, :], op=op)
            return t[:, 9, :]

        psum_t = [None] * UPC
        for u in range(UPC):
            for ch in range(NCHUNKS):
                slot = u * NCHUNKS + ch
                xs = xpool.tile([P, 2 * C + 1, WCHUNK], bf16, tag="xs",
                                name=f"xs{slot}")
                nc.sync.dma_start(xs[:, 0:C, :], x_d[u, ch])
                nc.gpsimd.memset(xs[:, 2 * C, :], 1.0)
                nc.scalar.activation(xs[:, C:2 * C, :], xs[:, 0:C, :],
                                     Act.Square)

                m = tree(xs[:, 0:C, :], f"m{slot}")
                # E one-hot, written directly in matmul-grouped layout:
                # eq[p, g, c*WG + j] = (xb[p, c, g*WG+j] == m[p, g*WG+j]),
                # so each group's stationary is one contiguous 76-wide run.
                NG = WCHUNK // WG
                eq = epool.tile([P, NG, MROWS], bf16, tag="eq", name=f"eq{slot}")
                eq_v = eq[:].rearrange("p g (c j) -> p c g j", c=C, j=WG)
                xb_v = xs[:, 0:C, :].rearrange("p c (g j) -> p c g j", j=WG)
                m_b = (m.rearrange("p (g j) -> p g j", j=WG)
                       .unsqueeze(1).broadcast_to([P, C, NG, WG]))
                nc.vector.tensor_tensor(out=eq_v, in0=xb_v, in1=m_b,
                                        op=Alu.is_equal)

                if ch == 0:
                    psum_t[u] = ppool.tile([MROWS, NCOLS], f32, tag="ps",
                                           name=f"ps{u}")
                for g in range(NG):
                    rhs = xs[:, :, WG * g:WG * (g + 1)].transpose([0, 2, 1])
                    nc.tensor.matmul(
                        psum_t[u][:, :], eq[:, g, :], rhs,
                        start=(ch == 0 and g == 0),
                        stop=(ch == NCHUNKS - 1 and g == NG - 1),
                        skip_group_check=True,
                    )

            sb = spool.tile([MROWS, NCOLS], f32, tag="sb", name=f"sb{u}")
            nc.scalar.copy(out=sb[:], in_=psum_t[u][:, :])
            nc.sync.dma_start(out_d[u], sb[:])

    nc.compile()
    return nc


def _get_nc():
    if "nc" not in _CACHE:
        _CACHE["nc"] = _build_nc()
    return _CACHE["nc"]


def _make_shards(logits):
    xb = np.asarray(logits).astype(ml_dtypes.bfloat16, copy=False)
    shards = []
    for k in range(NCORES):
        units = [UNITS[UPC * k + i] for i in range(UPC)]
        arr = np.stack(
            [xb[n, :, s * P:(s + 1) * P, :] for (n, s) in units]
        )                                            # (UPC, C, P, W)
        arr = arr.reshape(UPC, C, P, NCHUNKS, WCHUNK)
        arr = arr.transpose(0, 3, 2, 1, 4)           # (UPC, NCH, P, C, WC)
        shards.append(np.ascontiguousarray(arr))
    return shards


def _finish(results):
    S1 = np.zeros((N, C), dtype=np.float64)
    S2 = np.zeros((N, C), dtype=np.float64)
    cnt = np.zeros((N, C), dtype=np.float64)
    for k in range(NCORES):
        arr = np.asarray(results[k]["stats"], dtype=np.float64)
        for u in range(UPC):
            n, _s = UNITS[UPC * k + u]
            blk = arr[u].reshape(C, WG, WG, NQ)
            diag = np.diagonal(blk, axis1=1, axis2=2)     # (C, NQ, WG)
            S1[n] += diag[:, 0:C, :].sum(axis=(1, 2))
            S2[n] += diag[:, C:2 * C, :].sum(axis=(1, 2))
            cnt[n] += diag[:, 2 * C, :].sum(axis=1)
    K = np.maximum(cnt, 1.0) * C
    sq_dev = np.maximum(S2 - S1 * S1 / K, 0.0)
    norms = np.where(cnt > 0, np.sqrt(sq_dev), 0.0)
    loss = norms.mean(axis=0).sum()
    return np.array(loss, dtype=np.float32)


def kernel(**inputs):
    _ensure_ntff_hook()
    from concourse.bass_utils import run_bass_kernel_spmd

    logits = np.asarray(inputs["logits"])
    assert logits.shape == (N, C, H, W), logits.shape
    nc = _get_nc()
    shards = _make_shards(logits)
    in_maps = [{"x": shards[k]} for k in range(NCORES)]
    res = run_bass_kernel_spmd(nc, in_maps, list(range(NCORES)))
    return _finish(res.results)


# revision 14
# speedup vs baseline: 1.8788x; 1.8788x over previous
"""CenterLoss (segment_reduce) Trainium2 kernel — TensorE segment-sum version.

Math (faithful to the reference):
  preds = argmax_c logits[n, c, h, w]          (softmax is monotone -> skip it)
  per (n, cls): cnt = #pixels with preds==cls,
                S1 = sum over those pixels of sum_c x,
                S2 = sum over those pixels of sum_c x^2
  K = max(cnt,1)*C; sq_dev = max(S2 - S1^2/K, 0)
  loss = sum_cls mean_n( cnt>0 ? sqrt(sq_dev) : 0 )

Device strategy (8 cores, data-parallel over 16 units = (n, H-slab of 128)):
  Host pre-casts logits to bf16 (halves HBM traffic; bf16-argmax ties touch
  ~0.7% of pixels and shift the loss by ~4e-3 rel — within the 2e-2 gate)
  and pre-arranges each chunk in a matmul-grouped layout
      xs[p, g, q, j]:  g = w-group of 4, q = 39 cols [xb(19)|sq(19)|one], j = w%4
  so that each group's moving operand is one contiguous 156-elem run
  (multi-free-dim moving APs stream ~4x slower on the PE).
    ScalarE:  Square(xb) -> xs[:, :, 19:38, :]
    GpSimd:   ones / memsets
    DVE:      m = max over c (bf16 pairwise tree, 2x mode; only level 1
              reads the grouped layout), then the one-hot
              E[p, g, c*4+j] = (xb == m) via ONE broadcast tensor_tensor,
              written directly in grouped form (contiguous 76-wide runs).
    TensorE:  per group, matmul(psum += E_g(128x76).T @ xs_g(128x156)),
              PSUM-accumulated over the whole unit (256 matmuls).  Rows are
              (c, jm), cols are (q, jn); only the jm == jn entries are used,
              giving per class the per-channel sums of xb and x^2 + count.
  The per-unit (76,156) PSUM is copied to SBUF (ScalarE) and DMA'd out;
  host extracts the diagonal blocks and applies the final sqrt/mean formula.

  Perf history on trn2 via axon: all-DVE baseline ~250-300us (57
  scalar_tensor_tensor passes/chunk); per-w-column matmuls (2048 tiny
  LDW+MM pairs, x3 col-tiled) ~139us; this grouped version cuts the PE
  stream to 512 (LDW 76 + MM N=156) pairs ~ 34 ns/w-col.
"""

import numpy as np
import ml_dtypes


def _ensure_ntff_hook():
    """bass_utils' trace path imports antenv.axon_hooks, which this image
    lacks.  Install a shim backed by trn_agent_boot's ctypes hook so a
    BASS_TRACE=1 environment doesn't crash the run (and tracing works)."""
    import sys
    import types

    try:
        import antenv.axon_hooks  # noqa: F401
        return
    except ImportError:
        pass
    try:
        from trn_agent_boot.trn_boot import _ntff_profile_via_ctypes

        hook = _ntff_profile_via_ctypes("/opt/axon/libaxon_pjrt.so")
    except Exception:
        hook = None
    mod = types.ModuleType("antenv.axon_hooks")
    mod.get_axon_ntff_profile_hook = lambda: hook
    mod.set_axon_ntff_profile_hook = lambda h: None
    sys.modules["antenv.axon_hooks"] = mod

N, C, H, W = 4, 19, 512, 1024
NCORES = 8
SLABS = 4                 # H split into 4 slabs of 128 partitions
P = H // SLABS            # 128
UNITS = [(n, s) for n in range(N) for s in range(SLABS)]   # 16 units
UPC = len(UNITS) // NCORES                                  # 2 units per core
WCHUNK = 512
NCHUNKS = W // WCHUNK
WG = 4                    # w-columns fused per matmul group
NG = WCHUNK // WG         # 128 groups per chunk
NQ = 2 * C + 1            # 39 xs columns: [xb-chans | sq-chans | count]
MROWS = C * WG            # 76 psum partitions, m = c*WG + jm
NCOLS = NQ * WG           # 156 psum columns, n = q*WG + jn; keep jm == jn

_CACHE = {}


def _build_nc():
    from contextlib import ExitStack

    import concourse.tile as tile
    from concourse import bacc, mybir

    f32 = mybir.dt.float32
    bf16 = mybir.dt.bfloat16
    Alu = mybir.AluOpType
    Act = mybir.ActivationFunctionType

    nc = bacc.Bacc("TRN2", target_bir_lowering=False, debug=False)
    # Host pre-arranges each core's shard as (unit, wchunk, p, g, c, j) bf16:
    # exactly the xb part of the grouped xs layout.
    x_d = nc.dram_tensor(
        "x", [UPC, NCHUNKS, P, NG, C, WG], bf16, kind="ExternalInput"
    ).ap()
    out_d = nc.dram_tensor(
        "stats", [UPC, MROWS, NCOLS], f32, kind="ExternalOutput"
    ).ap()

    with tile.TileContext(nc) as tc, ExitStack() as ctx:
        xpool = ctx.enter_context(tc.tile_pool(name="xs", bufs=2))
        epool = ctx.enter_context(tc.tile_pool(name="eq", bufs=2))
        tpool = ctx.enter_context(tc.tile_pool(name="tree", bufs=2))
        ppool = ctx.enter_context(tc.tile_pool(name="psum", bufs=2, space="PSUM"))
        spool = ctx.enter_context(tc.tile_pool(name="sb_out", bufs=2))

        def tree(src_g, tag):
            """Pairwise max-reduce over c of the grouped xb view `src_g`
            (P, C, NG, WG).  Level 1 reads the grouped layout (runs of WG);
            later levels run on a private contiguous (P, rows, WCHUNK) tile
            at full 2x DVE rate.  Returns a contiguous (P, WCHUNK) AP whose
            w index is g*WG + j."""
            assert C == 19
            t = tpool.tile([P, 10, WCHUNK], bf16, tag="tree", name=f"tree_{tag}")
            tt = nc.vector.tensor_tensor
            op = Alu.max
            t9 = t[:, 0:9, :].rearrange("p r (g j) -> p r g j", j=WG)
            tt(out=t9, in0=src_g[:, 0:9], in1=src_g[:, 9:18], op=op)
            tt(out=t[:, 0:4, :], in0=t[:, 0:4, :], in1=t[:, 4:8, :], op=op)
            tt(out=t[:, 0:2, :], in0=t[:, 0:2, :], in1=t[:, 2:4, :], op=op)
            tt(out=t[:, 0, :], in0=t[:, 0, :], in1=t[:, 1, :], op=op)
            tt(out=t[:, 0, :], in0=t[:, 0, :], in1=t[:, 8, :], op=op)
            tt(out=t[:, 9, :].rearrange("p (g j) -> p g j", j=WG),
               in0=t[:, 0, :].rearrange("p (g j) -> p g j", j=WG),
               in1=src_g[:, 18], op=op)
            return t[:, 9, :]

        psum_t = [None] * UPC
        for u in range(UPC):
            for ch in range(NCHUNKS):
                slot = u * NCHUNKS + ch
                xs = xpool.tile([P, NG, NQ, WG], bf16, tag="xs",
                                name=f"xs{slot}")
                nc.sync.dma_start(xs[:, :, 0:C, :], x_d[u, ch])
                nc.gpsimd.memset(xs[:, :, 2 * C, :], 1.0)
                nc.scalar.activation(xs[:, :, C:2 * C, :], xs[:, :, 0:C, :],
                                     Act.Square)

                xb_g = xs[:, :, 0:C, :].transpose([0, 2, 1, 3])  # (P,C,NG,WG)
                m = tree(xb_g, f"m{slot}")
                # E one-hot, written directly in matmul-grouped layout:
                # eq[p, g, c*WG + j] = (xb[p, g, c, j] == m[p, g*WG+j])
                eq = epool.tile([P, NG, MROWS], bf16, tag="eq", name=f"eq{slot}")
                eq_v = eq[:].rearrange("p g (c j) -> p c g j", c=C, j=WG)
                m_b = (m.rearrange("p (g j) -> p g j", j=WG)
                       .unsqueeze(1).broadcast_to([P, C, NG, WG]))
                nc.vector.tensor_tensor(out=eq_v, in0=xb_g, in1=m_b,
                                        op=Alu.is_equal)

                if ch == 0:
                    psum_t[u] = ppool.tile([MROWS, NCOLS], f32, tag="ps",
                                           name=f"ps{u}")
                xs_f = xs[:].rearrange("p g q j -> p g (q j)")
                for g in range(NG):
                    nc.tensor.matmul(
                        psum_t[u][:, :], eq[:, g, :], xs_f[:, g, :],
                        start=(ch == 0 and g == 0),
                        stop=(ch == NCHUNKS - 1 and g == NG - 1),
                        skip_group_check=True,
                    )

            sb = spool.tile([MROWS, NCOLS], f32, tag="sb", name=f"sb{u}")
            nc.scalar.copy(out=sb[:], in_=psum_t[u][:, :])
            nc.sync.dma_start(out_d[u], sb[:])

    nc.compile()
    return nc


def _get_nc():
    if "nc" not in _CACHE:
        _CACHE["nc"] = _build_nc()
    return _CACHE["nc"]


def _make_shards(logits):
    xb = np.asarray(logits).astype(ml_dtypes.bfloat16, copy=False)
    shards = []
    for k in range(NCORES):
        units = [UNITS[UPC * k + i] for i in range(UPC)]
        arr = np.stack(
            [xb[n, :, s * P:(s + 1) * P, :] for (n, s) in units]
        )                                            # (UPC, C, P, W)
        arr = arr.reshape(UPC, C, P, NCHUNKS, NG, WG)
        arr = arr.transpose(0, 3, 2, 4, 1, 5)        # (UPC, NCH, P, NG, C, WG)
        shards.append(np.ascontiguousarray(arr))
    return shards


def _finish(results):
    S1 = np.zeros((N, C), dtype=np.float64)
    S2 = np.zeros((N, C), dtype=np.float64)
    cnt = np.zeros((N, C), dtype=np.float64)
    for k in range(NCORES):
        arr = np.asarray(results[k]["stats"], dtype=np.float64)
        for u in range(UPC):
            n, _s = UNITS[UPC * k + u]
            blk = arr[u].reshape(C, WG, NQ, WG)           # [c, jm, q, jn]
            diag = np.diagonal(blk, axis1=1, axis2=3)     # (C, NQ, WG)
            S1[n] += diag[:, 0:C, :].sum(axis=(1, 2))
            S2[n] += diag[:, C:2 * C, :].sum(axis=(1, 2))
            cnt[n] += diag[:, 2 * C, :].sum(axis=1)
    K = np.maximum(cnt, 1.0) * C
    sq_dev = np.maximum(S2 - S1 * S1 / K, 0.0)
    norms = np.where(cnt > 0, np.sqrt(sq_dev), 0.0)
    loss = norms.mean(axis=0).sum()
    return np.array(loss, dtype=np.float32)


def kernel(**inputs):
    _ensure_ntff_hook()
    from concourse.bass_utils import run_bass_kernel_spmd

    logits = np.asarray(inputs["logits"])
    assert logits.shape == (N, C, H, W), logits.shape
    nc = _get_nc()
    shards = _make_shards(logits)
    in_maps = [{"x": shards[k]} for k in range(NCORES)]
    res = run_bass_kernel_spmd(nc, in_maps, list(range(NCORES)))
    return _finish(res.results)
